# revision 31
# baseline (speedup 1.0000x reference)
"""Causal self-attention (B=4, T=2048, C=1024, nh=16) on 8 Trainium2 NeuronCores.

Sharding: tensor-parallel over heads (2 heads/core). Each core receives:
  - xT:  [1024, 8192]  transposed activations (replicated)
  - wq:  [1024, 384]   Wqkv columns for its 2 heads, ordered [q0|q1|k0|k1|v0|v1]
  - wp:  [128, 1024]   Wproj rows for its 2 heads' channels
and produces y_part (full-shape partial, host-summed), plus its kT/vT shards.

Device pipeline per batch b:
  A) qkvT = wq_shard.T @ xT  -> Q^T,K^T,V^T stacked [128, 2048] (f32r)
  B) V~ = PE-transpose(V^T) padded to [128(k),128] with a ones column (rowsum trick)
  C) per 512-q block j: S^T[k,q] = K^T_stat.T @ Q^T_mov; causal mask on diagonal
     tiles; P^T = exp(S^T/8) (ACT, f32r); O_aug^T[65,q] += V~^T @ P^T
  D) transpose O_aug -> divide rows by rowsum col -> transpose back -> O^T
  E) y_part[t, :] = O^T_stat.T @ Wproj_mov  (single 128-contraction)

All matmuls run in float32r (1 cyc/row at N=512; ~1.6e-4 rel err).
"""

import numpy as np
from contextlib import ExitStack

import concourse.bass as bass
import concourse.tile as tile
from concourse import bacc, mybir
from concourse.bass_utils import run_bass_kernel_spmd
from concourse.masks import make_identity

F32 = mybir.dt.float32
F32R = mybir.dt.float32r
EXP = mybir.ActivationFunctionType.Exp
LN = mybir.ActivationFunctionType.Ln

N_CORES = 8
B, T, C = 4, 2048, 1024
NH, HD = 16, 64
HPC = NH // N_CORES          # heads per core = 2
BT = B * T                   # 8192

_CACHE = {}


def _build():
    nc = bacc.Bacc("TRN2", target_bir_lowering=False, debug=False, num_devices=N_CORES)
    xT = nc.dram_tensor("xT", [C, BT], F32, kind="ExternalInput").ap()
    wq = nc.dram_tensor("wq", [C, 3 * HPC * HD], F32, kind="ExternalInput").ap()
    wp = nc.dram_tensor("wp", [HPC * HD, C], F32, kind="ExternalInput").ap()
    y_out = nc.dram_tensor("y_part", [BT, C], F32, kind="ExternalOutput").ap()
    kT_out = nc.dram_tensor("kT", [B, HPC * HD, T], F32, kind="ExternalOutput").ap()
    vT_out = nc.dram_tensor("vT", [B, HPC * HD, T], F32, kind="ExternalOutput").ap()

    with tile.TileContext(nc) as tc, ExitStack() as ctx:
        const = ctx.enter_context(tc.tile_pool(name="const", bufs=1))
        bpool = ctx.enter_context(tc.tile_pool(name="bpool", bufs=2))
        xpool = ctx.enter_context(tc.tile_pool(name="xpool", bufs=2))
        ppool = ctx.enter_context(tc.tile_pool(name="ppool", bufs=4))
        spool = ctx.enter_context(tc.tile_pool(name="spool", bufs=8))
        ypool = ctx.enter_context(tc.tile_pool(name="ypool", bufs=2))
        psQ = ctx.enter_context(tc.tile_pool(name="psQ", bufs=2, space="PSUM"))
        psS = ctx.enter_context(tc.tile_pool(name="psS", bufs=3, space="PSUM"))
        psO = ctx.enter_context(tc.tile_pool(name="psO", bufs=3, space="PSUM"))

        # --- constants ---
        ident = const.tile([128, 128], F32)
        make_identity(nc, ident[:])
        identr = const.tile([128, 128], F32R)
        nc.scalar.copy(out=identr[:], in_=ident[:])
        # additive causal masks for the 4 diagonal offsets of S^T [k=128, q=512],
        # duplicated across the head dim so one DVE add covers the merged pair
        masks = const.tile([128, 4, 512], mybir.dt.bfloat16)
        for o in range(4):
            nc.gpsimd.memset(masks[:, o, :], 0.0)
            nc.gpsimd.affine_select(
                out=masks[:, o, :], in_=masks[:, o, :],
                compare_op=mybir.AluOpType.is_ge,
                fill=-1e10, base=-128 * o,
                pattern=[[1, 512]], channel_multiplier=-1,
            )
        # pad block for V~ columns 64..127: col 64 = 1 (rowsum), rest 0
        padcol = const.tile([128, 64], F32)
        nc.any.memset(padcol[:], 0.0)
        nc.any.memset(padcol[:, 0:1], 1.0)
        # weights (DMA straight into f32r tiles)
        wq_sb = const.tile([128, 8, 3 * 128], F32R)
        for kc in range(8):
            nc.sync.dma_start(out=wq_sb[:, kc, :], in_=wq[kc * 128:(kc + 1) * 128, :].bitcast(F32R))
        wp_sb = const.tile([128, C], F32R)
        nc.sync.dma_start(out=wp_sb[:], in_=wp[:].bitcast(F32R))

        for b in range(B):
            QT = bpool.tile([128, T], F32R, tag="QT")
            KT = bpool.tile([128, T], F32R, tag="KT")
            VT = bpool.tile([128, T], F32R, tag="VT")
            OT = bpool.tile([128, T], F32R, tag="OT")
            vtil = bpool.tile([128, HPC, 16, 128], F32R, tag="vtil")
            rsums = [spool.tile([1, T], F32, tag=f"rsums{h}", name=f"rsums{b}_{h}", bufs=2)
                     for h in range(HPC)]

            # --- A: qkvT for this batch ---
            for tb in range(4):
                t0 = b * T + tb * 512
                xt = xpool.tile([128, 8, 512], F32R, tag="xt")
                for kc in range(8):
                    nc.sync.dma_start(
                        out=xt[:, kc, :],
                        in_=xT[kc * 128:(kc + 1) * 128, t0:t0 + 512].bitcast(F32R),
                    )
                for g, dest in enumerate((QT, KT, VT)):
                    pq = psQ.tile([128, 512], F32, tag="q")
                    for kc in range(8):
                        nc.tensor.matmul(
                            pq[:], wq_sb[:, kc, g * 128:(g + 1) * 128], xt[:, kc, :],
                            start=(kc == 0), stop=(kc == 7),
                        )
                    nc.vector.tensor_copy(out=dest[:, tb * 512:tb * 512 + 512], in_=pq[:])
                # V~ build for this tb's 4 token-chunks (interleaved with stage-A
                # matmuls so the PE-transposes never cluster long enough to
                # re-throttle the HAM clock gate)
                for tc_ in range(4 * tb, 4 * tb + 4):
                    pt = psQ.tile([128, 128], F32R, tag="q")
                    nc.tensor.transpose(pt[:], VT[:, tc_ * 128:(tc_ + 1) * 128], identr[:])
                    for h in range(HPC):
                        nc.vector.tensor_copy(out=vtil[:, h, tc_, 0:64], in_=pt[:, h * 64:h * 64 + 64])
                        nc.vector.tensor_copy(out=vtil[:, h, tc_, 64:128], in_=padcol[:])
            nc.sync.dma_start(out=kT_out[b], in_=KT[:].bitcast(F32))
            nc.sync.dma_start(out=vT_out[b], in_=VT[:].bitcast(F32))

            # --- C: attention per 512-q block ---
            for j in range(4):
                oaug = [psO.tile([128, 512], F32, tag="o", name=f"oaug{b}_{j}_{h}") for h in range(HPC)]
                nkc = 4 * j + 4
                # Software-pipelined by one kc step: the PV pair for kc-1 is
                # emitted AFTER the S pair for kc, so in priority order the PE
                # runs S_h0,S_h1 back-to-back (base partitions 0/64 -> the two
                # matmuls pack onto the two array halves concurrently) while
                # ACT computes exp(kc-1).
                pend = None  # (kc, q0, pT2)
                for kc in range(nkc):
                    # Diagonal tiles (kc >= 4j): columns q < 128*o are fully
                    # masked -> skip them in S, exp and PV. Keep N >= 256 so
                    # f32r stays at full rate.
                    o = kc - 4 * j
                    q0 = min(128 * o, 256) if o >= 0 else 0
                    spss = []
                    for h in range(HPC):
                        sp = psS.tile([128, 512], F32, tag="s", name=f"sps{b}_{j}_{kc}_{h}")
                        nc.tensor.matmul(
                            sp[:, q0:512],
                            KT[h * 64:h * 64 + 64, kc * 128:(kc + 1) * 128],
                            QT[h * 64:h * 64 + 64, j * 512 + q0:(j + 1) * 512],
                            start=True, stop=True,
                        )
                        spss.append(sp)
                    pTs = []
                    for h in range(HPC):
                        if o >= 0:
                            nc.vector.tensor_add(
                                spss[h][:, q0:512], spss[h][:, q0:512], masks[:, o, q0:512]
                            )
                        pT = ppool.tile([128, 512], F32R, tag="pT", name=f"pT{b}_{j}_{kc}_{h}")
                        nc.scalar.activation(
                            out=pT[:, q0:512], in_=spss[h][:, q0:512], func=EXP, scale=0.125
                        )
                        pTs.append(pT)
                    if pend is not None:
                        pkc, pq0, ppTs = pend
                        for h in range(HPC):
                            nc.tensor.matmul(
                                oaug[h][:, pq0:512], vtil[:, h, pkc, :], ppTs[h][:, pq0:512],
                                start=(pkc == 0), stop=False,
                            )
                    pend = (kc, q0, pTs)
                pkc, pq0, ppTs = pend
                for h in range(HPC):
                    nc.tensor.matmul(
                        oaug[h][:, pq0:512], vtil[:, h, pkc, :], ppTs[h][:, pq0:512],
                        start=(pkc == 0), stop=True,
                    )

                # --- D (per j): stash unnormalized O^T and the rowsum row ---
                for h in range(HPC):
                    nc.vector.tensor_copy(
                        out=OT[h * 64:h * 64 + 64, j * 512:(j + 1) * 512],
                        in_=oaug[h][0:64, :],
                    )
                    nc.scalar.copy(
                        out=rsums[h][:, j * 512:(j + 1) * 512], in_=oaug[h][64:65, :]
                    )
                # --- D (per half-batch): reciprocal via in-place ACT ln+exp(-x),
                # gpsimd partition-broadcast, one in-place DVE mul per head;
                # split in column halves so proj can start before the last j ---
                if j in (1, 3):
                    c0, c1 = (0, 1024) if j == 1 else (1024, 2048)
                    for h in range(HPC):
                        sl = slice(c0, c1)
                        nc.scalar.activation(out=rsums[h][:, sl], in_=rsums[h][:, sl], func=LN)
                        nc.scalar.activation(out=rsums[h][:, sl], in_=rsums[h][:, sl], func=EXP, scale=-1.0)
                        rcpb = spool.tile([128, 1024], F32, tag="rcpb", name=f"rcpb{b}_{j}_{h}", bufs=2)
                        nc.gpsimd.partition_broadcast(rcpb[:], rsums[h][:, sl])
                        nc.vector.tensor_mul(
                            OT[h * 64:h * 64 + 64, sl], OT[h * 64:h * 64 + 64, sl],
                            rcpb[h * 64:h * 64 + 64, :],
                        )

            # --- E: projection (partial y) ---
            for tch in range(16):
                for co in range(2):
                    yp = psS.tile([128, 512], F32, tag="s")
                    nc.tensor.matmul(
                        yp[:],
                        OT[:, tch * 128:(tch + 1) * 128],
                        wp_sb[:, co * 512:(co + 1) * 512],
                        start=True, stop=True,
                    )
                    ysb = ypool.tile([128, 512], F32, tag="ysb")
                    nc.vector.tensor_copy(out=ysb[:], in_=yp[:])
                    nc.sync.dma_start(
                        out=y_out[b * T + tch * 128: b * T + (tch + 1) * 128,
                                  co * 512:(co + 1) * 512],
                        in_=ysb[:],
                    )

    nc.compile()
    return nc


def get_nc():
    if "nc" not in _CACHE:
        _CACHE["nc"] = _build()
    return _CACHE["nc"]


def make_in_maps(x, Wqkv, Wproj):
    x = np.asarray(x, dtype=np.float32)
    Wqkv = np.asarray(Wqkv, dtype=np.float32)
    Wproj = np.asarray(Wproj, dtype=np.float32)
    xT = np.ascontiguousarray(x.reshape(BT, C).T)
    in_maps = []
    for c in range(N_CORES):
        h0, h1 = HPC * c, HPC * c + 1
        cols = []
        for g in range(3):  # q, k, v
            for h in (h0, h1):
                cols.append(Wqkv[:, g * C + h * HD:g * C + (h + 1) * HD])
        wq_shard = np.ascontiguousarray(np.concatenate(cols, axis=1))
        wp_shard = np.ascontiguousarray(Wproj[c * 128:(c + 1) * 128, :])
        in_maps.append({"xT": xT, "wq": wq_shard, "wp": wp_shard})
    return in_maps


def assemble(results):
    y = np.zeros((BT, C), dtype=np.float32)
    for r in results:
        y += r["y_part"]
    y = y.reshape(B, T, C)
    # kT/vT: per core [B, 2*64, T] -> k [B, NH, T, HD]
    kT = np.stack([r["kT"] for r in results], axis=0)  # [8, B, 128, T]
    vT = np.stack([r["vT"] for r in results], axis=0)
    def unshard(aT):
        a = aT.reshape(N_CORES, B, HPC, HD, T)          # [8, B, 2, 64, T]
        a = a.transpose(1, 0, 2, 4, 3)                  # [B, 8, 2, T, 64]
        return np.ascontiguousarray(a.reshape(B, NH, T, HD))
    return y, unshard(kT), unshard(vT)


def kernel(x, Wqkv, Wproj):
    nc = get_nc()
    in_maps = make_in_maps(x, Wqkv, Wproj)
    res = run_bass_kernel_spmd(nc, in_maps, list(range(N_CORES)))
    return assemble(res.results)


if __name__ == "__main__":
    rng = np.random.default_rng(0)
    x = rng.standard_normal((B, T, C), dtype=np.float32)
    Wqkv = (rng.standard_normal((C, 3 * C)) * 0.02).astype(np.float32)
    Wproj = (rng.standard_normal((C, C)) * 0.02).astype(np.float32)
    y, k, v = kernel(x, Wqkv, Wproj)
    print("shapes:", y.shape, k.shape, v.shape)


# revision 32
# speedup vs baseline: 1.0845x; 1.0845x over previous
"""Causal self-attention (B=4, T=2048, C=1024, nh=16) on 8 Trainium2 NeuronCores.

Sharding: tensor-parallel over heads (2 heads/core). Each core receives:
  - xT:  [1024, 8192]  transposed activations (replicated)
  - wq:  [1024, 384]   Wqkv columns for its 2 heads, ordered [q0|q1|k0|k1|v0|v1]
  - wp:  [128, 1024]   Wproj rows for its 2 heads' channels
and produces y_part (full-shape partial, host-summed), plus its kT/vT shards.

Device pipeline per batch b:
  A) qkvT = wq_shard.T @ xT  -> Q^T,K^T,V^T stacked [128, 2048] (f32r)
  B) V~ = PE-transpose(V^T) padded to [128(k),128] with a ones column (rowsum trick)
  C) per 512-q block j: S^T[k,q] = K^T_stat.T @ Q^T_mov; causal mask on diagonal
     tiles; P^T = exp(S^T/8) (ACT, f32r); O_aug^T[65,q] += V~^T @ P^T
  D) transpose O_aug -> divide rows by rowsum col -> transpose back -> O^T
  E) y_part[t, :] = O^T_stat.T @ Wproj_mov  (single 128-contraction)

All matmuls run in float32r (1 cyc/row at N=512; ~1.6e-4 rel err).
"""

import numpy as np
from contextlib import ExitStack

import concourse.bass as bass
import concourse.tile as tile
from concourse import bacc, mybir
from concourse.bass_utils import run_bass_kernel_spmd
from concourse.masks import make_identity

F32 = mybir.dt.float32
F32R = mybir.dt.float32r
EXP = mybir.ActivationFunctionType.Exp
LN = mybir.ActivationFunctionType.Ln

N_CORES = 8
B, T, C = 4, 2048, 1024
NH, HD = 16, 64
HPC = NH // N_CORES          # heads per core = 2
BT = B * T                   # 8192

_CACHE = {}


def _build():
    nc = bacc.Bacc("TRN2", target_bir_lowering=False, debug=False, num_devices=N_CORES)
    xT = nc.dram_tensor("xT", [C, BT], F32, kind="ExternalInput").ap()
    wq = nc.dram_tensor("wq", [C, 3 * HPC * HD], F32, kind="ExternalInput").ap()
    wp = nc.dram_tensor("wp", [HPC * HD, C], F32, kind="ExternalInput").ap()
    y_out = nc.dram_tensor("y_part", [BT, C], F32, kind="ExternalOutput").ap()
    kT_out = nc.dram_tensor("kT", [B, HPC * HD, T], F32, kind="ExternalOutput").ap()
    vT_out = nc.dram_tensor("vT", [B, HPC * HD, T], F32, kind="ExternalOutput").ap()

    with tile.TileContext(nc) as tc, ExitStack() as ctx:
        const = ctx.enter_context(tc.tile_pool(name="const", bufs=1))
        bpool = ctx.enter_context(tc.tile_pool(name="bpool", bufs=2))
        xpool = ctx.enter_context(tc.tile_pool(name="xpool", bufs=2))
        ppool = ctx.enter_context(tc.tile_pool(name="ppool", bufs=4))
        spool = ctx.enter_context(tc.tile_pool(name="spool", bufs=8))
        ypool = ctx.enter_context(tc.tile_pool(name="ypool", bufs=2))
        psQ = ctx.enter_context(tc.tile_pool(name="psQ", bufs=2, space="PSUM"))
        psS = ctx.enter_context(tc.tile_pool(name="psS", bufs=3, space="PSUM"))
        psO = ctx.enter_context(tc.tile_pool(name="psO", bufs=3, space="PSUM"))

        # --- constants ---
        ident = const.tile([128, 128], F32)
        make_identity(nc, ident[:])
        identr = const.tile([128, 128], F32R)
        nc.scalar.copy(out=identr[:], in_=ident[:])
        # additive causal masks for the 4 diagonal offsets of S^T [k=128, q=512],
        # duplicated across the head dim so one DVE add covers the merged pair
        masks = const.tile([128, 4, 512], mybir.dt.bfloat16)
        for o in range(4):
            nc.gpsimd.memset(masks[:, o, :], 0.0)
            nc.gpsimd.affine_select(
                out=masks[:, o, :], in_=masks[:, o, :],
                compare_op=mybir.AluOpType.is_ge,
                fill=-1e10, base=-128 * o,
                pattern=[[1, 512]], channel_multiplier=-1,
            )
        # pad block for V~ columns 64..127: col 64 = 1 (rowsum), rest 0
        padcol = const.tile([128, 64], F32)
        nc.any.memset(padcol[:], 0.0)
        nc.any.memset(padcol[:, 0:1], 1.0)
        # weights (DMA straight into f32r tiles)
        wq_sb = const.tile([128, 8, 3 * 128], F32R)
        for kc in range(8):
            nc.sync.dma_start(out=wq_sb[:, kc, :], in_=wq[kc * 128:(kc + 1) * 128, :].bitcast(F32R))
        wp_sb = const.tile([128, C], F32R)
        nc.sync.dma_start(out=wp_sb[:], in_=wp[:].bitcast(F32R))

        for b in range(B):
            QT = bpool.tile([128, T], F32R, tag="QT")
            KT = bpool.tile([128, T], F32R, tag="KT")
            VT = bpool.tile([128, T], F32R, tag="VT")
            OT = bpool.tile([128, T], F32R, tag="OT")
            vtil = bpool.tile([128, HPC, 16, 128], F32R, tag="vtil")
            rsums = [spool.tile([1, T], F32, tag=f"rsums{h}", name=f"rsums{b}_{h}", bufs=2)
                     for h in range(HPC)]

            # --- A: qkvT for this batch ---
            for tb in range(4):
                t0 = b * T + tb * 512
                xt = xpool.tile([128, 8, 512], F32R, tag="xt")
                for kc in range(8):
                    nc.sync.dma_start(
                        out=xt[:, kc, :],
                        in_=xT[kc * 128:(kc + 1) * 128, t0:t0 + 512].bitcast(F32R),
                    )
                for g, dest in enumerate((QT, KT, VT)):
                    pq = psQ.tile([128, 512], F32, tag="q")
                    for kc in range(8):
                        nc.tensor.matmul(
                            pq[:], wq_sb[:, kc, g * 128:(g + 1) * 128], xt[:, kc, :],
                            start=(kc == 0), stop=(kc == 7),
                        )
                    nc.any.tensor_copy(out=dest[:, tb * 512:tb * 512 + 512], in_=pq[:])
                # V~ build for this tb's 4 token-chunks (interleaved with stage-A
                # matmuls so the PE-transposes never cluster long enough to
                # re-throttle the HAM clock gate)
                for tc_ in range(4 * tb, 4 * tb + 4):
                    pt = psQ.tile([128, 128], F32R, tag="q")
                    nc.tensor.transpose(pt[:], VT[:, tc_ * 128:(tc_ + 1) * 128], identr[:])
                    for h in range(HPC):
                        nc.any.tensor_copy(out=vtil[:, h, tc_, 0:64], in_=pt[:, h * 64:h * 64 + 64])
                        nc.vector.tensor_copy(out=vtil[:, h, tc_, 64:128], in_=padcol[:])
            nc.sync.dma_start(out=kT_out[b], in_=KT[:].bitcast(F32))
            nc.sync.dma_start(out=vT_out[b], in_=VT[:].bitcast(F32))

            # --- C: attention per 512-q block ---
            for j in range(4):
                oaug = [psO.tile([128, 512], F32, tag="o", name=f"oaug{b}_{j}_{h}") for h in range(HPC)]
                nkc = 4 * j + 4
                # Software-pipelined by one kc step: the PV pair for kc-1 is
                # emitted AFTER the S pair for kc, so in priority order the PE
                # runs S_h0,S_h1 back-to-back (base partitions 0/64 -> the two
                # matmuls pack onto the two array halves concurrently) while
                # ACT computes exp(kc-1).
                pend = None  # (kc, q0, pT2)
                for kc in range(nkc):
                    # Diagonal tiles (kc >= 4j): columns q < 128*o are fully
                    # masked -> skip them in S, exp and PV. Keep N >= 256 so
                    # f32r stays at full rate.
                    o = kc - 4 * j
                    q0 = min(128 * o, 256) if o >= 0 else 0
                    spss = []
                    for h in range(HPC):
                        sp = psS.tile([128, 512], F32, tag="s", name=f"sps{b}_{j}_{kc}_{h}")
                        nc.tensor.matmul(
                            sp[:, q0:512],
                            KT[h * 64:h * 64 + 64, kc * 128:(kc + 1) * 128],
                            QT[h * 64:h * 64 + 64, j * 512 + q0:(j + 1) * 512],
                            start=True, stop=True,
                        )
                        spss.append(sp)
                    pTs = []
                    for h in range(HPC):
                        if o >= 0:
                            m0, m1 = (128 * o, 128 * o + 128) if o < 3 else (256, 512)
                            nc.vector.tensor_add(
                                spss[h][:, m0:m1], spss[h][:, m0:m1], masks[:, o, m0:m1]
                            )
                        pT = ppool.tile([128, 512], F32R, tag="pT", name=f"pT{b}_{j}_{kc}_{h}")
                        nc.scalar.activation(
                            out=pT[:, q0:512], in_=spss[h][:, q0:512], func=EXP, scale=0.125
                        )
                        pTs.append(pT)
                    if pend is not None:
                        pkc, pq0, ppTs = pend
                        for h in range(HPC):
                            nc.tensor.matmul(
                                oaug[h][:, pq0:512], vtil[:, h, pkc, :], ppTs[h][:, pq0:512],
                                start=(pkc == 0), stop=False,
                            )
                    pend = (kc, q0, pTs)
                pkc, pq0, ppTs = pend
                for h in range(HPC):
                    nc.tensor.matmul(
                        oaug[h][:, pq0:512], vtil[:, h, pkc, :], ppTs[h][:, pq0:512],
                        start=(pkc == 0), stop=True,
                    )

                # --- D (per j): stash unnormalized O^T and the rowsum row ---
                for h in range(HPC):
                    nc.vector.tensor_copy(
                        out=OT[h * 64:h * 64 + 64, j * 512:(j + 1) * 512],
                        in_=oaug[h][0:64, :],
                    )
                    nc.scalar.copy(
                        out=rsums[h][:, j * 512:(j + 1) * 512], in_=oaug[h][64:65, :]
                    )
                # --- D (per half-batch): reciprocal via in-place ACT ln+exp(-x),
                # gpsimd partition-broadcast, one in-place DVE mul per head;
                # split in column halves so proj can start before the last j ---
                if j in (1, 3):
                    c0, c1 = (0, 1024) if j == 1 else (1024, 2048)
                    for h in range(HPC):
                        sl = slice(c0, c1)
                        nc.scalar.activation(out=rsums[h][:, sl], in_=rsums[h][:, sl], func=LN)
                        nc.scalar.activation(out=rsums[h][:, sl], in_=rsums[h][:, sl], func=EXP, scale=-1.0)
                        rcpb = spool.tile([128, 1024], F32, tag="rcpb", name=f"rcpb{b}_{j}_{h}", bufs=2)
                        nc.gpsimd.partition_broadcast(rcpb[:], rsums[h][:, sl])
                        nc.vector.tensor_mul(
                            OT[h * 64:h * 64 + 64, sl], OT[h * 64:h * 64 + 64, sl],
                            rcpb[h * 64:h * 64 + 64, :],
                        )

            # --- E: projection (partial y) ---
            for tch in range(16):
                for co in range(2):
                    yp = psS.tile([128, 512], F32, tag="s")
                    nc.tensor.matmul(
                        yp[:],
                        OT[:, tch * 128:(tch + 1) * 128],
                        wp_sb[:, co * 512:(co + 1) * 512],
                        start=True, stop=True,
                    )
                    ysb = ypool.tile([128, 512], F32, tag="ysb")
                    nc.any.tensor_copy(out=ysb[:], in_=yp[:])
                    nc.sync.dma_start(
                        out=y_out[b * T + tch * 128: b * T + (tch + 1) * 128,
                                  co * 512:(co + 1) * 512],
                        in_=ysb[:],
                    )

    nc.compile()
    return nc


def get_nc():
    if "nc" not in _CACHE:
        _CACHE["nc"] = _build()
    return _CACHE["nc"]


def make_in_maps(x, Wqkv, Wproj):
    x = np.asarray(x, dtype=np.float32)
    Wqkv = np.asarray(Wqkv, dtype=np.float32)
    Wproj = np.asarray(Wproj, dtype=np.float32)
    xT = np.ascontiguousarray(x.reshape(BT, C).T)
    in_maps = []
    for c in range(N_CORES):
        h0, h1 = HPC * c, HPC * c + 1
        cols = []
        for g in range(3):  # q, k, v
            for h in (h0, h1):
                cols.append(Wqkv[:, g * C + h * HD:g * C + (h + 1) * HD])
        wq_shard = np.ascontiguousarray(np.concatenate(cols, axis=1))
        wp_shard = np.ascontiguousarray(Wproj[c * 128:(c + 1) * 128, :])
        in_maps.append({"xT": xT, "wq": wq_shard, "wp": wp_shard})
    return in_maps


def assemble(results):
    y = np.zeros((BT, C), dtype=np.float32)
    for r in results:
        y += r["y_part"]
    y = y.reshape(B, T, C)
    # kT/vT: per core [B, 2*64, T] -> k [B, NH, T, HD]
    kT = np.stack([r["kT"] for r in results], axis=0)  # [8, B, 128, T]
    vT = np.stack([r["vT"] for r in results], axis=0)
    def unshard(aT):
        a = aT.reshape(N_CORES, B, HPC, HD, T)          # [8, B, 2, 64, T]
        a = a.transpose(1, 0, 2, 4, 3)                  # [B, 8, 2, T, 64]
        return np.ascontiguousarray(a.reshape(B, NH, T, HD))
    return y, unshard(kT), unshard(vT)


def kernel(x, Wqkv, Wproj):
    nc = get_nc()
    in_maps = make_in_maps(x, Wqkv, Wproj)
    res = run_bass_kernel_spmd(nc, in_maps, list(range(N_CORES)))
    return assemble(res.results)


if __name__ == "__main__":
    rng = np.random.default_rng(0)
    x = rng.standard_normal((B, T, C), dtype=np.float32)
    Wqkv = (rng.standard_normal((C, 3 * C)) * 0.02).astype(np.float32)
    Wproj = (rng.standard_normal((C, C)) * 0.02).astype(np.float32)
    y, k, v = kernel(x, Wqkv, Wproj)
    print("shapes:", y.shape, k.shape, v.shape)


# revision 33
# speedup vs baseline: 1.0856x; 1.0010x over previous
"""Causal self-attention (B=4, T=2048, C=1024, nh=16) on 8 Trainium2 NeuronCores.

Sharding: tensor-parallel over heads (2 heads/core). Each core receives:
  - xT:  [1024, 8192]  transposed activations (replicated)
  - wq:  [1024, 384]   Wqkv columns for its 2 heads, ordered [q0|q1|k0|k1|v0|v1]
  - wp:  [128, 1024]   Wproj rows for its 2 heads' channels
and produces y_part (full-shape partial, host-summed), plus its kT/vT shards.

Device pipeline per batch b:
  A) qkvT = wq_shard.T @ xT  -> Q^T,K^T,V^T stacked [128, 2048] (f32r)
  B) V~ = PE-transpose(V^T) padded to [128(k),128] with a ones column (rowsum trick)
  C) per 512-q block j: S^T[k,q] = K^T_stat.T @ Q^T_mov; causal mask on diagonal
     tiles; P^T = exp(S^T/8) (ACT, f32r); O_aug^T[65,q] += V~^T @ P^T
  D) transpose O_aug -> divide rows by rowsum col -> transpose back -> O^T
  E) y_part[t, :] = O^T_stat.T @ Wproj_mov  (single 128-contraction)

All matmuls run in float32r (1 cyc/row at N=512; ~1.6e-4 rel err).
"""

import numpy as np
from contextlib import ExitStack

import concourse.bass as bass
import concourse.tile as tile
from concourse import bacc, mybir
from concourse.bass_utils import run_bass_kernel_spmd
from concourse.masks import make_identity

F32 = mybir.dt.float32
F32R = mybir.dt.float32r
EXP = mybir.ActivationFunctionType.Exp
LN = mybir.ActivationFunctionType.Ln

N_CORES = 8
B, T, C = 4, 2048, 1024
NH, HD = 16, 64
HPC = NH // N_CORES          # heads per core = 2
BT = B * T                   # 8192

_CACHE = {}


def _build():
    nc = bacc.Bacc("TRN2", target_bir_lowering=False, debug=False, num_devices=N_CORES)
    xT = nc.dram_tensor("xT", [C, BT], F32, kind="ExternalInput").ap()
    wq = nc.dram_tensor("wq", [C, 3 * HPC * HD], F32, kind="ExternalInput").ap()
    wp = nc.dram_tensor("wp", [HPC * HD, C], F32, kind="ExternalInput").ap()
    y_out = nc.dram_tensor("y_part", [BT, C], F32, kind="ExternalOutput").ap()
    kT_out = nc.dram_tensor("kT", [B, HPC * HD, T], F32, kind="ExternalOutput").ap()
    vT_out = nc.dram_tensor("vT", [B, HPC * HD, T], F32, kind="ExternalOutput").ap()

    with tile.TileContext(nc) as tc, ExitStack() as ctx:
        const = ctx.enter_context(tc.tile_pool(name="const", bufs=1))
        bpool = ctx.enter_context(tc.tile_pool(name="bpool", bufs=2))
        xpool = ctx.enter_context(tc.tile_pool(name="xpool", bufs=2))
        ppool = ctx.enter_context(tc.tile_pool(name="ppool", bufs=4))
        spool = ctx.enter_context(tc.tile_pool(name="spool", bufs=8))
        ypool = ctx.enter_context(tc.tile_pool(name="ypool", bufs=2))
        psQ = ctx.enter_context(tc.tile_pool(name="psQ", bufs=2, space="PSUM"))
        psS = ctx.enter_context(tc.tile_pool(name="psS", bufs=3, space="PSUM"))
        psO = ctx.enter_context(tc.tile_pool(name="psO", bufs=3, space="PSUM"))

        # --- constants ---
        ident = const.tile([128, 128], F32)
        make_identity(nc, ident[:])
        identr = const.tile([128, 128], F32R)
        nc.scalar.copy(out=identr[:], in_=ident[:])
        # additive causal masks for the 4 diagonal offsets of S^T [k=128, q=512],
        # duplicated across the head dim so one DVE add covers the merged pair
        masks = const.tile([128, 4, 512], mybir.dt.bfloat16)
        for o in range(4):
            nc.gpsimd.memset(masks[:, o, :], 0.0)
            nc.gpsimd.affine_select(
                out=masks[:, o, :], in_=masks[:, o, :],
                compare_op=mybir.AluOpType.is_ge,
                fill=-1e10, base=-128 * o,
                pattern=[[1, 512]], channel_multiplier=-1,
            )
        # pad block for V~ columns 64..127: col 64 = 1 (rowsum), rest 0
        padcol = const.tile([128, 64], F32)
        nc.any.memset(padcol[:], 0.0)
        nc.any.memset(padcol[:, 0:1], 1.0)
        # weights (DMA straight into f32r tiles)
        wq_sb = const.tile([128, 8, 3 * 128], F32R)
        for kc in range(8):
            nc.sync.dma_start(out=wq_sb[:, kc, :], in_=wq[kc * 128:(kc + 1) * 128, :].bitcast(F32R))
        wp_sb = const.tile([128, C], F32R)
        nc.sync.dma_start(out=wp_sb[:], in_=wp[:].bitcast(F32R))

        for b in range(B):
            QT = bpool.tile([128, T], F32R, tag="QT")
            KT = bpool.tile([128, T], F32R, tag="KT")
            VT = bpool.tile([128, T], F32R, tag="VT")
            OT = bpool.tile([128, T], F32R, tag="OT")
            vtil = bpool.tile([128, HPC, 16, 128], F32R, tag="vtil")
            rsums = [spool.tile([1, T], F32, tag=f"rsums{h}", name=f"rsums{b}_{h}", bufs=2)
                     for h in range(HPC)]

            # --- A: qkvT for this batch ---
            for tb in range(4):
                t0 = b * T + tb * 512
                xt = xpool.tile([128, 8, 512], F32R, tag="xt")
                for kc in range(8):
                    nc.sync.dma_start(
                        out=xt[:, kc, :],
                        in_=xT[kc * 128:(kc + 1) * 128, t0:t0 + 512].bitcast(F32R),
                    )
                for g, dest in enumerate((QT, KT, VT)):
                    pq = psQ.tile([128, 512], F32, tag="q")
                    for kc in range(8):
                        nc.tensor.matmul(
                            pq[:], wq_sb[:, kc, g * 128:(g + 1) * 128], xt[:, kc, :],
                            start=(kc == 0), stop=(kc == 7),
                        )
                    nc.any.tensor_copy(out=dest[:, tb * 512:tb * 512 + 512], in_=pq[:])
                # V~ build for this tb's 4 token-chunks (interleaved with stage-A
                # matmuls so the PE-transposes never cluster long enough to
                # re-throttle the HAM clock gate)
                for tc_ in range(4 * tb, 4 * tb + 4):
                    pt = psQ.tile([128, 128], F32R, tag="q")
                    nc.tensor.transpose(pt[:], VT[:, tc_ * 128:(tc_ + 1) * 128], identr[:])
                    for h in range(HPC):
                        nc.any.tensor_copy(out=vtil[:, h, tc_, 0:64], in_=pt[:, h * 64:h * 64 + 64])
                        nc.vector.tensor_copy(out=vtil[:, h, tc_, 64:128], in_=padcol[:])
            nc.sync.dma_start(out=kT_out[b], in_=KT[:].bitcast(F32))
            nc.sync.dma_start(out=vT_out[b], in_=VT[:].bitcast(F32))

            # --- C: attention per 512-q block ---
            for j in range(4):
                oaug = [psO.tile([128, 512], F32, tag="o", name=f"oaug{b}_{j}_{h}") for h in range(HPC)]
                nkc = 4 * j + 4
                # Software-pipelined by one kc step: the PV pair for kc-1 is
                # emitted AFTER the S pair for kc, so in priority order the PE
                # runs S_h0,S_h1 back-to-back (base partitions 0/64 -> the two
                # matmuls pack onto the two array halves concurrently) while
                # ACT computes exp(kc-1).
                pend = None  # (kc, q0, pT2)
                for kc in range(nkc):
                    # Diagonal tiles (kc >= 4j): columns q < 128*o are fully
                    # masked -> skip them in S, exp and PV. Keep N >= 256 so
                    # f32r stays at full rate.
                    o = kc - 4 * j
                    q0 = min(128 * o, 256) if o >= 0 else 0
                    spss = []
                    for h in range(HPC):
                        sp = psS.tile([128, 512], F32, tag="s", name=f"sps{b}_{j}_{kc}_{h}")
                        nc.tensor.matmul(
                            sp[:, q0:512],
                            KT[h * 64:h * 64 + 64, kc * 128:(kc + 1) * 128],
                            QT[h * 64:h * 64 + 64, j * 512 + q0:(j + 1) * 512],
                            start=True, stop=True,
                        )
                        spss.append(sp)
                    pTs = []
                    for h in range(HPC):
                        if o >= 0:
                            m0, m1 = (128 * o, 128 * o + 128) if o < 3 else (256, 512)
                            nc.vector.tensor_add(
                                spss[h][:, m0:m1], spss[h][:, m0:m1], masks[:, o, m0:m1]
                            )
                        pT = ppool.tile([128, 512], F32R, tag="pT", name=f"pT{b}_{j}_{kc}_{h}")
                        nc.scalar.activation(
                            out=pT[:, q0:512], in_=spss[h][:, q0:512], func=EXP, scale=0.125
                        )
                        pTs.append(pT)
                    if pend is not None:
                        pkc, pq0, ppTs = pend
                        for h in range(HPC):
                            nc.tensor.matmul(
                                oaug[h][:, pq0:512], vtil[:, h, pkc, :], ppTs[h][:, pq0:512],
                                start=(pkc == 0), stop=False,
                            )
                    pend = (kc, q0, pTs)
                pkc, pq0, ppTs = pend
                for h in range(HPC):
                    nc.tensor.matmul(
                        oaug[h][:, pq0:512], vtil[:, h, pkc, :], ppTs[h][:, pq0:512],
                        start=(pkc == 0), stop=True,
                    )

                # --- D (per j): stash unnormalized O^T and the rowsum row ---
                for h in range(HPC):
                    nc.vector.tensor_copy(
                        out=OT[h * 64:h * 64 + 64, j * 512:(j + 1) * 512],
                        in_=oaug[h][0:64, :],
                    )
                    nc.vector.tensor_copy(
                        out=rsums[h][:, j * 512:(j + 1) * 512], in_=oaug[h][64:65, :]
                    )
                # --- D (per half-batch): reciprocal via in-place ACT ln+exp(-x),
                # gpsimd partition-broadcast, one in-place DVE mul per head;
                # split in column halves so proj can start before the last j ---
                if j in (1, 3):
                    c0, c1 = (0, 1024) if j == 1 else (1024, 2048)
                    sl = slice(c0, c1)
                    # group LN ops then EXP ops: each Ln<->Exp switch costs a
                    # ~1.3us activation-table reload on ACT
                    for h in range(HPC):
                        nc.scalar.activation(out=rsums[h][:, sl], in_=rsums[h][:, sl], func=LN)
                    for h in range(HPC):
                        nc.scalar.activation(out=rsums[h][:, sl], in_=rsums[h][:, sl], func=EXP, scale=-1.0)
                    for h in range(HPC):
                        rcpb = spool.tile([128, 1024], F32, tag="rcpb", name=f"rcpb{b}_{j}_{h}", bufs=2)
                        nc.gpsimd.partition_broadcast(rcpb[:], rsums[h][:, sl])
                        nc.vector.tensor_mul(
                            OT[h * 64:h * 64 + 64, sl], OT[h * 64:h * 64 + 64, sl],
                            rcpb[h * 64:h * 64 + 64, :],
                        )

            # --- E: projection (partial y) ---
            for tch in range(16):
                for co in range(2):
                    yp = psS.tile([128, 512], F32, tag="s")
                    nc.tensor.matmul(
                        yp[:],
                        OT[:, tch * 128:(tch + 1) * 128],
                        wp_sb[:, co * 512:(co + 1) * 512],
                        start=True, stop=True,
                    )
                    ysb = ypool.tile([128, 512], F32, tag="ysb")
                    nc.any.tensor_copy(out=ysb[:], in_=yp[:])
                    nc.sync.dma_start(
                        out=y_out[b * T + tch * 128: b * T + (tch + 1) * 128,
                                  co * 512:(co + 1) * 512],
                        in_=ysb[:],
                    )

    nc.compile()
    return nc


def get_nc():
    if "nc" not in _CACHE:
        _CACHE["nc"] = _build()
    return _CACHE["nc"]


def make_in_maps(x, Wqkv, Wproj):
    x = np.asarray(x, dtype=np.float32)
    Wqkv = np.asarray(Wqkv, dtype=np.float32)
    Wproj = np.asarray(Wproj, dtype=np.float32)
    xT = np.ascontiguousarray(x.reshape(BT, C).T)
    in_maps = []
    for c in range(N_CORES):
        h0, h1 = HPC * c, HPC * c + 1
        cols = []
        for g in range(3):  # q, k, v
            for h in (h0, h1):
                cols.append(Wqkv[:, g * C + h * HD:g * C + (h + 1) * HD])
        wq_shard = np.ascontiguousarray(np.concatenate(cols, axis=1))
        wp_shard = np.ascontiguousarray(Wproj[c * 128:(c + 1) * 128, :])
        in_maps.append({"xT": xT, "wq": wq_shard, "wp": wp_shard})
    return in_maps


def assemble(results):
    y = np.zeros((BT, C), dtype=np.float32)
    for r in results:
        y += r["y_part"]
    y = y.reshape(B, T, C)
    # kT/vT: per core [B, 2*64, T] -> k [B, NH, T, HD]
    kT = np.stack([r["kT"] for r in results], axis=0)  # [8, B, 128, T]
    vT = np.stack([r["vT"] for r in results], axis=0)
    def unshard(aT):
        a = aT.reshape(N_CORES, B, HPC, HD, T)          # [8, B, 2, 64, T]
        a = a.transpose(1, 0, 2, 4, 3)                  # [B, 8, 2, T, 64]
        return np.ascontiguousarray(a.reshape(B, NH, T, HD))
    return y, unshard(kT), unshard(vT)


def kernel(x, Wqkv, Wproj):
    nc = get_nc()
    in_maps = make_in_maps(x, Wqkv, Wproj)
    res = run_bass_kernel_spmd(nc, in_maps, list(range(N_CORES)))
    return assemble(res.results)


if __name__ == "__main__":
    rng = np.random.default_rng(0)
    x = rng.standard_normal((B, T, C), dtype=np.float32)
    Wqkv = (rng.standard_normal((C, 3 * C)) * 0.02).astype(np.float32)
    Wproj = (rng.standard_normal((C, C)) * 0.02).astype(np.float32)
    y, k, v = kernel(x, Wqkv, Wproj)
    print("shapes:", y.shape, k.shape, v.shape)
